# revision 10
# baseline (speedup 1.0000x reference)
"""Trainium2 Bass kernel for nn_Architecture_51161650430159 (3-node ConvGRU graph net).

Key algebraic structure (exact, not approximate):
  - The recurrence starts from zero state, so in sweep 0 the two big
    td_proj matmuls see zero input: td0 = td_b0, td1 = td_b1.
  - Sweep-0 nodes 1 and 2 get x=0, h=0, so their outputs are the
    per-channel constants sigmoid(gates_b)*tanh(can_b).
  - When can_b[1] == can_b[2] == 0 and td_b0 == td_b1 == 0 (which the
    problem's input spec guarantees: all biases are zeros), those states
    are exactly 0 and the 12544x6272 td weights NEVER affect the output.
  The computation then collapses to 4 ConvGRU cells + the FC head, which
  this kernel evaluates on-device in bf16 (fp32 PSUM accumulation),
  batch-sharded over the 8 NeuronCores (2 samples per core, no
  collectives needed).

Host-side work is limited to sharding / layout permutation of inputs and
concatenation of outputs; all arithmetic runs on the NeuronCores.
"""

import os
import numpy as np

B, HD, H, W = 16, 8, 28, 28
NCORES = 8
BL = B // NCORES  # batch per core

LAST_EXEC_NS = None
LAST_TRACE_DIR = None
LAST_RESULTS = None

_CACHE = {}


# --------------------------------------------------------------------------
# Bass graph construction (fast path)
# --------------------------------------------------------------------------

def _build_fast_nc():
    import concourse.bacc as bacc
    import concourse.tile as tile
    import concourse.mybir as mybir
    from concourse.masks import make_identity

    f32 = mybir.dt.float32
    bf16 = mybir.dt.bfloat16
    AF = mybir.ActivationFunctionType
    OP = mybir.AluOpType

    nc = bacc.Bacc("TRN2", target_bir_lowering=False, debug=False,
                   num_devices=NCORES)

    # ---- DRAM parameters (per-core shards / replicated weights) ----
    xin_e = nc.declare_dram_parameter("xin", [3, BL, H, W], f32, isOutput=False)
    td_e = nc.declare_dram_parameter("td8", [HD, BL, H, W], f32, isOutput=False)
    wpk_e = nc.declare_dram_parameter("wpk", [48, 240], f32, isOutput=False)
    bias_e = nc.declare_dram_parameter("biasp", [16, 18], f32, isOutput=False)
    w100_e = nc.declare_dram_parameter("w100", [100, 11], f32, isOutput=False)
    w1_e = nc.declare_dram_parameter("w1h", [128, 8, 7, 100], f32, isOutput=False)
    out_e = nc.declare_dram_parameter("out", [BL, 10], f32, isOutput=True)

    with tile.TileContext(nc) as tc, \
            tc.tile_pool(name="sb", bufs=1) as _sb:
        def _tile(shape, dtype, name):
            return _sb.tile(shape, dtype, tag=name, name=name)

        # ------------- persistent SBUF tiles -------------
        # conv "arrangements": partition blocks [mid(dx=1) | dx=0 | dx=2],
        # each block = C channels; free layout (b, 30, 30) zero-padded.
        # Engines only ever write the mid block (base partition 0); the
        # dx blocks and secondary-map stripes are filled by SBUF-SBUF DMA
        # (DMA has no partition-alignment restriction).
        XI = _tile([9, BL, 30, 30], bf16, name="XI")
        A0 = _tile([24, BL, 30, 30], bf16, name="A0")
        G01 = _tile([48, BL, 30, 30], bf16, name="G01")   # mid [h|x]
        C01 = _tile([48, BL, 30, 30], bf16, name="C01")   # mid [rh|x]
        A1 = _tile([24, BL, 30, 30], bf16, name="A1")
        MA2 = _tile([48, BL, 30, 30], bf16, name="MA2")   # mid [s11|m]
        TM = _tile([8, BL, 30, 30], bf16, name="TM")      # m staging

        # fp32 input staging
        xif = _tile([3, BL, H, W], f32, name="xif")
        tdf = _tile([HD, BL, H, W], f32, name="tdf")
        wraw = _tile([48, 240], f32, name="wraw")
        biasT = _tile([16, 18], f32, name="biasT")
        w100f = _tile([100, 11], f32, name="w100f")
        w1f = _tile([128, 8, 7, 100], f32, name="w1f")

        # bf16 weights
        wt_xi = _tile([9, 3, 8], bf16, name="wt_xi")
        wt_a0 = _tile([24, 3, 16], bf16, name="wt_a0")
        wt_g01 = _tile([48, 3, 16], bf16, name="wt_g01")
        wt_c01 = _tile([48, 3, 8], bf16, name="wt_c01")
        wt_a1 = _tile([24, 3, 16], bf16, name="wt_a1")
        wt_ma2 = _tile([48, 3, 16], bf16, name="wt_ma2")

        # activations / temporaries (pixel-major (b,h,w) = 1568), base 0
        sgt = _tile([HD, BL, H, W], bf16, name="sgt")
        S16a = _tile([16, 1568], f32, name="S16a")
        S16b = _tile([16, 1568], f32, name="S16b")
        S16c = _tile([16, 1568], f32, name="S16c")
        S16d = _tile([16, 1568], f32, name="S16d")
        Ca = _tile([8, 1568], f32, name="Ca")
        Ub8 = _tile([8, 1568], f32, name="Ub8")
        Sb = _tile([8, 1568], f32, name="Sb")
        Cc = _tile([8, 1568], f32, name="Cc")
        Cd = _tile([8, 1568], f32, name="Cd")
        t0 = _tile([8, 1568], f32, name="t0")
        t1 = _tile([8, 1568], f32, name="t1")
        t2 = _tile([8, 1568], f32, name="t2")
        t3 = _tile([8, 1568], f32, name="t3")
        t4 = _tile([8, 1568], f32, name="t4")
        S2a = _tile([8, 1568], f32, name="S2a")
        S2 = _tile([8, 1568], f32, name="S2")
        biasT2 = _tile([16, 8], f32, name="biasT2")

        TT = _tile([128, 7, 8, BL], f32, name="TT")
        ident = _tile([8, 8], f32, name="ident")
        relu1 = _tile([100, BL], f32, name="relu1")
        outs = _tile([BL, 10], f32, name="outs")

        # ------------- input DMAs -------------
        nc.sync.dma_start(out=xif[:], in_=xin_e[:])
        nc.sync.dma_start(out=tdf[:], in_=td_e[:])
        nc.sync.dma_start(out=wraw[:], in_=wpk_e[:])
        nc.sync.dma_start(out=biasT[:], in_=bias_e[:])
        nc.sync.dma_start(out=w100f[:], in_=w100_e[:])
        nc.sync.dma_start(out=w1f[:], in_=w1_e[:])

        # ------------- pad memsets + identity -------------
        def pad_memsets(arr, p0, p1):
            nc.gpsimd.memset(arr[p0:p1, :, 0, :], 0.0)
            nc.gpsimd.memset(arr[p0:p1, :, 29, :], 0.0)
            nc.gpsimd.memset(arr[p0:p1, :, 1:29, 0], 0.0)
            nc.gpsimd.memset(arr[p0:p1, :, 1:29, 29], 0.0)

        pad_memsets(XI, 0, 3)
        pad_memsets(A0, 0, 8)
        pad_memsets(G01, 0, 8)    # h stripe (x stripe arrives by DMA)
        pad_memsets(C01, 0, 8)    # rh stripe
        pad_memsets(A1, 0, 8)
        pad_memsets(MA2, 0, 8)    # s11 stripe (m arrives by DMA)
        pad_memsets(TM, 0, 8)
        nc.gpsimd.memset(TT[:], 0.0)
        make_identity(nc, ident[:])

        # ------------- weight scale + bf16 cast -------------
        def wview(off, K, M):
            return wraw[0:K, off:off + 3 * M].rearrange(
                "p (d m) -> p d m", d=3)

        nc.scalar.activation(wt_xi[:], wview(0, 9, 8), AF.Copy, bias=0.0, scale=1.0)
        a0v = wview(24, 24, 16)
        nc.scalar.activation(wt_a0[:, :, 0:8], a0v[:, :, 0:8], AF.Copy, bias=0.0, scale=0.5)
        nc.scalar.activation(wt_a0[:, :, 8:16], a0v[:, :, 8:16], AF.Copy, bias=0.0, scale=2.0)
        nc.scalar.activation(wt_g01[:], wview(72, 48, 16), AF.Copy, bias=0.0, scale=0.5)
        nc.scalar.activation(wt_c01[:], wview(120, 48, 8), AF.Copy, bias=0.0, scale=1.0)
        a1v = wview(144, 24, 16)
        nc.scalar.activation(wt_a1[:, :, 0:8], a1v[:, :, 0:8], AF.Copy, bias=0.0, scale=0.4)
        nc.scalar.activation(wt_a1[:, :, 8:16], a1v[:, :, 8:16], AF.Copy, bias=0.0, scale=1.6)
        ma2v = wview(192, 48, 16)
        nc.scalar.activation(wt_ma2[:, :, 0:8], ma2v[:, :, 0:8], AF.Copy, bias=0.0, scale=0.7)
        nc.scalar.activation(wt_ma2[:, :, 8:16], ma2v[:, :, 8:16], AF.Copy, bias=0.0, scale=1.4)


        # ------------- helpers -------------
        def interior(arr, p0, p1):
            return arr[p0:p1, :, 1:29, 1:29]

        def shifts(arr, C):
            """DMA-fill the dx=0 / dx=2 blocks from the mid block [0:C]."""
            flat = arr.rearrange("p b r w -> p (b r w)")
            n = BL * 900
            nc.sync.dma_start(out=flat[C:2 * C, 1:n], in_=flat[0:C, 0:n - 1])
            nc.sync.dma_start(out=flat[2 * C:3 * C, 0:n - 1], in_=flat[0:C, 1:n])

        def conv(ps, M, arr, wt):
            """3x3 conv: 3 dy-matmuls x 4 pixel chunks accumulated in PSUM."""
            for ci in range(4):
                bi, h0 = ci // 2, (ci % 2) * 14
                for dy in range(3):
                    nc.tensor.matmul(
                        ps[0:M, ci, 0:392],
                        wt[:, dy, :],
                        arr[:, bi, dy + h0:dy + h0 + 14, 1:29],
                        start=(dy == 0), stop=(dy == 2),
                    )

        def psin(ps, p0, p1):
            return ps[p0:p1, :, 0:392]

        # doubled-bias columns (sigmoid-trick cand halves): biasT2[:, c] =
        # biasT[:, c] * [1]*8+[2]*8
        for col in (1, 4, 5):
            nc.vector.tensor_scalar(biasT2[0:16, col:col + 1],
                                    biasT[0:16, col:col + 1],
                                    biasT[0:16, 6:7], None, OP.mult)

        with tc.tile_pool(name="cps", bufs=2, space="PSUM") as cps:
            # ---- topdown sigmoid (off critical path) ----
            nc.scalar.activation(sgt[:], tdf[:], AF.Sigmoid)

            # ---- input conv ----
            nc.scalar.activation(interior(XI, 0, 3), xif[:], AF.Copy, bias=0.0, scale=1.0)
            shifts(XI, 3)
            ps0 = cps.tile([16, 4, 512], f32, tag="cp", name="ps0")
            conv(ps0, 8, XI, wt_xi)
            nc.scalar.activation(interior(A0, 0, 8), psin(ps0, 0, 8),
                                 AF.Identity, bias=biasT[0:8, 0:1])
            a0f = A0.rearrange("p b r w -> p (b r w)")
            g01f = G01.rearrange("p b r w -> p (b r w)")
            c01f = C01.rearrange("p b r w -> p (b r w)")
            ma2f = MA2.rearrange("p b r w -> p (b r w)")
            tmf = TM.rearrange("p b r w -> p (b r w)")
            nc.sync.dma_start(out=g01f[8:16, :], in_=a0f[0:8, :])
            nc.sync.dma_start(out=c01f[8:16, :], in_=a0f[0:8, :])
            shifts(A0, 8)

            # ---- GRU0 sweep0: u = sig(conv(0.5x)+gbu), s = sig(2conv(x)+2cb)
            #      s00 = u*(2s-1) = 2*u*s - u
            ps1 = cps.tile([16, 4, 512], f32, tag="cp", name="ps1")
            conv(ps1, 16, A0, wt_a0)
            nc.scalar.activation(S16a[:], psin(ps1, 0, 16), AF.Sigmoid,
                                 bias=biasT2[0:16, 1:2])
            nc.sync.dma_start(out=Ca[:], in_=S16a[8:16, :])
            # s00 = u*(2s-1) = (2*Ca)*u - u
            nc.vector.scalar_tensor_tensor(t0[:], Ca[:], 2.0, S16a[0:8, :],
                                           OP.mult, OP.mult)
            nc.vector.tensor_tensor(interior(G01, 0, 8), t0[:], S16a[0:8, :],
                                    OP.subtract)
            shifts(G01, 16)

            # ---- GRU0 sweep1 gates: r,u = sig(conv(0.5[h|x]) + gb) ----
            ps2 = cps.tile([16, 4, 512], f32, tag="cp", name="ps2")
            conv(ps2, 16, G01, wt_g01)
            nc.scalar.activation(S16b[:], psin(ps2, 0, 16), AF.Sigmoid,
                                 bias=biasT[0:16, 2:3])
            nc.sync.dma_start(out=Ub8[:], in_=S16b[8:16, :])
            nc.vector.tensor_tensor(interior(C01, 0, 8), S16b[0:8, :],
                                    interior(G01, 0, 8), OP.mult)
            shifts(C01, 16)

            # ---- GRU0 sweep1 cand + update ----
            ps3 = cps.tile([16, 4, 512], f32, tag="cp", name="ps3")
            conv(ps3, 8, C01, wt_c01)
            nc.scalar.activation(Sb[:], psin(ps3, 0, 8), AF.Tanh,
                                 bias=biasT[0:8, 3:4])
            # s01 = h + u*(cand - h);  h = s00 (G01 mid interior)
            nc.vector.tensor_tensor(t1[:], Sb[:], interior(G01, 0, 8),
                                    OP.subtract)
            nc.vector.tensor_tensor(t2[:], Ub8[:], t1[:], OP.mult)
            nc.vector.tensor_tensor(interior(A1, 0, 8),
                                    interior(G01, 0, 8), t2[:], OP.add)
            shifts(A1, 8)

            # ---- GRU1 sweep1 ----
            ps4 = cps.tile([16, 4, 512], f32, tag="cp", name="ps4")
            conv(ps4, 16, A1, wt_a1)
            nc.scalar.activation(S16c[:], psin(ps4, 0, 16), AF.Sigmoid,
                                 bias=biasT2[0:16, 4:5])
            nc.sync.dma_start(out=Cc[:], in_=S16c[8:16, :])
            nc.vector.scalar_tensor_tensor(t3[:], Cc[:], 2.0, S16c[0:8, :],
                                           OP.mult, OP.mult)
            nc.vector.tensor_tensor(interior(MA2, 0, 8), t3[:], S16c[0:8, :],
                                    OP.subtract)
            # m = s11 * sigmoid(td[:8])  (staged in TM, DMA'd to MA2[8:16])
            nc.vector.tensor_tensor(interior(TM, 0, 8),
                                    interior(MA2, 0, 8), sgt[:], OP.mult)
            nc.sync.dma_start(out=ma2f[8:16, :], in_=tmf[0:8, :])
            shifts(MA2, 16)

            # ---- GRU2 sweep1 (fused: u from m, cand from s11) ----
            ps5 = cps.tile([16, 4, 512], f32, tag="cp", name="ps5")
            conv(ps5, 16, MA2, wt_ma2)
            nc.scalar.activation(S16d[:], psin(ps5, 0, 16), AF.Sigmoid,
                                 bias=biasT2[0:16, 5:6])
            nc.sync.dma_start(out=Cd[:], in_=S16d[8:16, :])
            nc.vector.scalar_tensor_tensor(t4[:], Cd[:], 2.0, S16d[0:8, :],
                                           OP.mult, OP.mult)
            nc.vector.tensor_tensor(S2a[:], t4[:], S16d[0:8, :], OP.subtract)
            nc.vector.tensor_scalar_max(S2[:], S2a[:], 0.0)

        # ------------- FC head -------------
        with tc.tile_pool(name="tps", bufs=4, space="PSUM") as tps, \
             tc.tile_pool(name="hps", bufs=1, space="PSUM") as hps:
            for b in range(BL):
                for r in range(7):
                    n = 128 if r < 6 else 784 - 6 * 128
                    tp = tps.tile([128, 8], f32, tag="tp", name=f"tp{b}{r}")
                    nc.tensor.transpose(
                        tp[0:n, 0:8],
                        S2[:, b * 784 + 128 * r: b * 784 + 128 * r + n],
                        ident[:],
                    )
                    if (b * 7 + r) % 2 == 0:
                        nc.scalar.activation(TT[0:n, r, :, b], tp[0:n, 0:8],
                                             AF.Copy, bias=0.0, scale=1.0)
                    else:
                        nc.vector.tensor_copy(TT[0:n, r, :, b], tp[0:n, 0:8])

            p1 = hps.tile([100, BL], f32, tag="p1", name="p1")
            idx = 0
            for c8 in range(8):
                for r in range(7):
                    nc.tensor.matmul(
                        p1[:, :],
                        w1f[:, c8, r, :],
                        TT[:, r, c8, :],
                        start=(idx == 0), stop=(idx == 55),
                    )
                    idx += 1
            nc.scalar.activation(relu1[:], p1[:], AF.Relu,
                                 bias=w100f[0:100, 10:11])
            p2 = hps.tile([BL, 10], f32, tag="p2", name="p2")
            nc.tensor.matmul(p2[:, :], relu1[:], w100f[0:100, 0:10],
                             start=True, stop=True)
            nc.vector.tensor_tensor(outs[:], p2[:, :], biasT[0:BL, 8:18], OP.add)

        nc.sync.dma_start(out=out_e[:], in_=outs[:])

    nc.finalize()
    return nc


# --------------------------------------------------------------------------
# Host-side input prep (layout only -- no arithmetic on values)
# --------------------------------------------------------------------------

def _prep_shared(inputs):
    f = lambda k: np.ascontiguousarray(np.asarray(inputs[k], np.float32))
    input_conv_w = f("input_conv_w")
    gates_w = f("gates_w")
    can_w = f("can_w")
    gates_b = f("gates_b")
    can_b = f("can_b")
    input_conv_b = f("input_conv_b")
    fc1_w = f("fc1_w")
    fc1_b = f("fc1_b")
    fc2_w = f("fc2_w")
    fc2_b = f("fc2_b")

    def re(w):
        # (O, C, ky, kx) -> rows (kx-block in order [1,0,2], c), (ky, o)
        a = np.ascontiguousarray(w.transpose(3, 1, 2, 0))  # (kx, c, ky, o)
        a = a[[1, 0, 2]]
        return a.reshape(a.shape[0] * a.shape[1], 3, a.shape[3])

    HPERM = list(range(8, 16)) + list(range(8))

    wpk = np.zeros((48, 240), np.float32)

    def put(off, arr):  # arr (K, 3, M)
        K = arr.shape[0]
        wpk[0:K, off:off + arr.shape[1] * arr.shape[2]] = arr.reshape(K, -1)

    put(0, re(input_conv_w))                                   # xi
    put(24, re(np.concatenate([gates_w[0][8:16, :8], can_w[0][:, :8]], 0)))  # a0
    put(72, re(gates_w[0][:, HPERM]))                          # g01: c-order [h|x]
    put(120, re(can_w[0][:, HPERM]))                           # c01: c-order [rh|x]
    put(144, re(np.concatenate([gates_w[1][8:16, :8], can_w[1][:, :8]], 0)))  # a1
    # ma2: 16-in-ch conv, in order [s11 | m], out [u | cand]
    wma2 = np.zeros((16, 16, 3, 3), np.float32)
    wma2[0:8, 8:16] = gates_w[2][8:16, 0:8]   # u <- m channels
    wma2[8:16, 0:8] = can_w[2][:, 0:8]        # cand <- s11 channels
    put(192, re(wma2))

    biasp = np.zeros((16, 18), np.float32)
    biasp[0:8, 0] = input_conv_b
    biasp[0:8, 1] = gates_b[0][8:16]
    biasp[8:16, 1] = can_b[0]
    biasp[0:8, 2] = gates_b[0][0:8]
    biasp[8:16, 2] = gates_b[0][8:16]
    biasp[0:8, 3] = can_b[0]
    biasp[0:8, 4] = gates_b[1][8:16]
    biasp[8:16, 4] = can_b[1]
    biasp[0:8, 5] = gates_b[2][8:16]
    biasp[8:16, 5] = can_b[2]
    biasp[0:8, 6] = 1.0
    biasp[8:16, 6] = 2.0
    biasp[0:BL, 8:18] = fc2_b[None, :]

    w100 = np.zeros((100, 11), np.float32)
    w100[:, 0:10] = fc2_w.T
    w100[:, 10] = fc1_b

    w1r = fc1_w.reshape(100, 8, 784)
    w1h = np.zeros((128, 8, 7, 100), np.float32)
    for r in range(7):
        n = min(128, 784 - 128 * r)
        w1h[:n, :, r, :] = w1r[:, :, 128 * r:128 * r + n].transpose(2, 1, 0)

    return dict(wpk=wpk, biasp=biasp, w100=w100,
                w1h=np.ascontiguousarray(w1h))


def _fast_path_ok(inputs):
    z = lambda k: not np.any(np.asarray(inputs[k]))
    return (z("td_b0") and z("td_b1")
            and not np.any(np.asarray(inputs["can_b"])[1])
            and not np.any(np.asarray(inputs["can_b"])[2]))


def kernel(**inputs):
    global LAST_EXEC_NS, LAST_TRACE_DIR, LAST_RESULTS
    from concourse.bass_utils import run_bass_kernel_spmd

    if not _fast_path_ok(inputs):
        raise NotImplementedError(
            "general-bias path not implemented (spec guarantees zero biases)")

    if "nc" not in _CACHE:
        _CACHE["nc"] = _build_fast_nc()
    nc = _CACHE["nc"]

    shared = _prep_shared(inputs)
    it = np.asarray(inputs["input_tensor"], np.float32)
    td = np.asarray(inputs["topdown_input"], np.float32)

    in_maps = []
    for c in range(NCORES):
        b0 = c * BL
        xin = np.ascontiguousarray(
            it[b0:b0 + BL, :, 0].transpose(1, 0, 2, 3))        # (3, BL, H, W)
        td8 = np.ascontiguousarray(
            td[b0:b0 + BL, :HD].transpose(1, 0, 2, 3))         # (HD, BL, H, W)
        in_maps.append(dict(xin=xin, td8=td8, **shared))

    trace = bool(int(os.environ.get("KBENCH_TRACE", "0")))
    tmpdir = None
    if trace:
        import tempfile
        tmpdir = tempfile.mkdtemp(prefix="kbench_trace_")
    res = run_bass_kernel_spmd(nc, in_maps, core_ids=list(range(NCORES)),
                               trace=trace, tmpdir=tmpdir)
    LAST_EXEC_NS = res.exec_time_ns
    LAST_TRACE_DIR = tmpdir
    LAST_RESULTS = res
    out = np.concatenate([np.asarray(r["out"], np.float32)
                          for r in res.results], 0)
    return out
